# revision 17
# baseline (speedup 1.0000x reference)
"""Galerkin linear attention (nn_Attention_7172595384411) on 8 TRN2 NeuronCores.

Math (reference):
    q = query @ Wq.T + bq   -> (h, N, dk)   [same for k, v]
    p_attn = einsum("hnd,hne->hde", k, v) / N          (h, dk, dk)
    x      = einsum("hnd,hde->hne", q, p_attn)         (h, N, dk)
    att    = x.transpose(0,2,1).reshape(N, 1, 128)

Key refactor: the projections fold out of the streaming passes.
    p_attn_h = (Wk_h C Wv_h^T + (Wk_h s_key) bv_h^T
                + bk_h (s_val^T Wv_h^T) + N bk_h bv_h^T) / N
with C = key^T value (128x128), s_key = key^T 1, s_val = 1^T value -- all
plain moments of the raw inputs, accumulated per-core over the N-shard and
AllReduced (66KB).  The output pass collapses to ONE fused matmul:
    Y[h*32+d, n] = sum_i BT[i, h*32+d] query[n, i] + c[h*32+d]
where BT[i, h*32+d] = sum_e Wq[h*32+e, i] p_attn_h[e, d] and
c[hd] = bq_h . p_attn_h[:, d].  The row-major flat layout of Y (128, N) is
exactly att_output.

Per core: stream key/value shard (8MB) for moments, AllReduce, tiny on-chip
algebra, stream query shard (4MB) -> Y shard (4MB).  ~16MB DMA/core.
"""

import numpy as np

N = 65536
D = 128
H = 4
DK = 32
NCORES = 8
NS = N // NCORES          # 8192 rows per core
P = 128                   # partitions
SUB = 4                   # 128-row subtiles per DMA tile
GROUP = P * SUB           # 512 rows per tile
TILES = NS // GROUP       # 16 tiles per phase

_cache = {}


def _build(repeat=1, use_f32r=True):
    import concourse.tile as tile
    from concourse import bacc, mybir
    from concourse.masks import make_identity

    f32 = mybir.dt.float32
    f32r = mybir.dt.float32r
    mmdt = f32r if use_f32r else f32

    def mm_cast(ap):
        # view an f32 DRAM AP as f32r for DMA into an f32r-typed tile
        return ap.bitcast(f32r) if use_f32r else ap

    nc = bacc.Bacc(
        "TRN2",
        target_bir_lowering=False,
        debug=False,
        enable_asserts=True,
        num_devices=NCORES,
    )

    xq = nc.dram_tensor("xq", [NS, D], f32, kind="ExternalInput").ap()
    xk = nc.dram_tensor("xk", [NS, D], f32, kind="ExternalInput").ap()
    xv = nc.dram_tensor("xv", [NS, D], f32, kind="ExternalInput").ap()
    wq = nc.dram_tensor("Wq", [D, D], f32, kind="ExternalInput").ap()
    wk = nc.dram_tensor("Wk", [D, D], f32, kind="ExternalInput").ap()
    wv = nc.dram_tensor("Wv", [D, D], f32, kind="ExternalInput").ap()
    bq = nc.dram_tensor("bq", [D], f32, kind="ExternalInput").ap()
    bk = nc.dram_tensor("bk", [D], f32, kind="ExternalInput").ap()
    bv = nc.dram_tensor("bv", [D], f32, kind="ExternalInput").ap()
    y_out = nc.dram_tensor("y", [D, NS], f32, kind="ExternalOutput").ap()
    ctx_out = nc.dram_tensor("ctx", [D, DK], f32, kind="ExternalOutput").ap()

    from contextlib import ExitStack

    with tile.TileContext(nc) as tc, ExitStack() as es:
        consts = es.enter_context(tc.tile_pool(name="consts", bufs=1))
        kv_pool = es.enter_context(tc.tile_pool(name="kv", bufs=3))
        q_pool = es.enter_context(tc.tile_pool(name="q", bufs=6))
        qt_pool = es.enter_context(tc.tile_pool(name="qt", bufs=3))
        ys_pool = es.enter_context(tc.tile_pool(name="ys", bufs=3))
        small = es.enter_context(tc.tile_pool(name="small", bufs=1))
        acc_psum = es.enter_context(
            tc.tile_pool(name="accp", bufs=1, space="PSUM"))
        pb_psum = es.enter_context(
            tc.tile_pool(name="pbp", bufs=2, space="PSUM"))
        qt_psum = es.enter_context(
            tc.tile_pool(name="qtp", bufs=2, space="PSUM"))
        y_psum = es.enter_context(
            tc.tile_pool(name="yp", bufs=2, space="PSUM"))
        dram = es.enter_context(tc.tile_pool(name="dram", bufs=1,
                                             space="DRAM"))
        if True:
            # ---- constants ----
            ident = consts.tile([P, P], f32)
            make_identity(nc, ident[:])
            ones_f32 = consts.tile([P, SUB, 2], f32)
            nc.gpsimd.memset(ones_f32[:], 1.0)
            if use_f32r:
                ones_col = consts.tile([P, 1], mmdt)
                nc.vector.tensor_copy(ones_col[:], ones_f32[:, 0, 0:1])
            else:
                ones_col = ones_f32[:, 0, 0:1]

            wq_nat = consts.tile([P, D], f32)
            nc.sync.dma_start(wq_nat[:], wq[:])
            wk_nat = consts.tile([P, D], f32)
            nc.sync.dma_start(wk_nat[:], wk[:])
            wv_nat = consts.tile([P, D], f32)
            nc.sync.dma_start(wv_nat[:], wv[:])
            bk_row = consts.tile([1, D], f32)
            nc.sync.dma_start(bk_row[:], bk[:])
            bv_row = consts.tile([1, D], f32)
            nc.sync.dma_start(bv_row[:], bv[:])
            bq_col = consts.tile([P, 1], f32)
            nc.sync.dma_start(bq_col[:], bq[:])
            bk_col = consts.tile([P, 1], f32)
            nc.sync.dma_start(bk_col[:], bk[:])

            # transposed weights (one-time)
            wkt_ps = pb_psum.tile([P, P], f32, tag="pb")
            nc.tensor.transpose(wkt_ps[:], wk_nat[:], ident[:])
            wkt = consts.tile([P, P], f32)
            nc.vector.tensor_copy(wkt[:], wkt_ps[:])
            wvt_ps = pb_psum.tile([P, P], f32, tag="pb")
            nc.tensor.transpose(wvt_ps[:], wv_nat[:], ident[:])
            wvt = consts.tile([P, P], f32)
            nc.vector.tensor_copy(wvt[:], wvt_ps[:])

            bkn = consts.tile([P, 1], f32)
            nc.scalar.mul(bkn[:], bk_col[:], float(N))

            for rep in range(repeat):
                last = rep == repeat - 1

                # ================= phase A: moments =================
                # CS[:, 0:128] = C = key^T value ; CS[:, 128] = s_key
                # SV[0, 0:128] = s_val
                # f32r: subtile widened to D+2 (even inner count required by
                # the fp32r dst-pattern ISA check) and doubled via a step-0
                # broadcast so the moving free dim is >=256 (full PE rate).
                vw = D + 2 if use_f32r else D + 1
                wide = 2 * vw if use_f32r else vw
                cs_ps = acc_psum.tile([P, wide], f32, tag="cs")
                sv_ps = acc_psum.tile([1, wide], f32, tag="sv")

                for t in range(TILES):
                    kt = kv_pool.tile([P, SUB, D], mmdt, tag="kt")
                    nc.sync.dma_start(
                        kt[:],
                        mm_cast(xk[t * GROUP:(t + 1) * GROUP, :].rearrange(
                            "(s p) i -> p s i", p=P)),
                    )
                    vt = kv_pool.tile([P, SUB, vw], mmdt, tag="vt")
                    nc.sync.dma_start(
                        vt[:, :, 0:D],
                        mm_cast(xv[t * GROUP:(t + 1) * GROUP, :].rearrange(
                            "(s p) i -> p s i", p=P)),
                    )
                    if use_f32r:
                        nc.vector.tensor_copy(vt[:, :, D:vw], ones_f32[:])
                    else:
                        nc.gpsimd.memset(vt[:, :, D:vw], 1.0)

                    for s in range(SUB):
                        first = t == 0 and s == 0
                        fin = t == TILES - 1 and s == SUB - 1
                        if use_f32r:
                            # moving free dim 260 (>=256) -> f32r full rate;
                            # cols 130:260 are a dup of 0:130.
                            rhs = vt[:, s, :].unsqueeze(1).broadcast_to(
                                [P, 2, vw])
                        else:
                            rhs = vt[:, s, :]
                        nc.tensor.matmul(
                            cs_ps[:],
                            kt[:, s, :],
                            rhs,
                            start=first,
                            stop=fin,
                        )
                        nc.tensor.matmul(
                            sv_ps[:],
                            ones_col[:],
                            rhs,
                            start=first,
                            stop=fin,
                        )

                # ================= AllReduce =================
                cc_in = dram.tile([130, D], f32, tag="ccin")
                cc_out = dram.tile([130, D], f32, addr_space="Shared",
                                   tag="ccout")
                cs_sb = small.tile([P, D + 1], f32, tag="cssb", bufs=2)
                nc.vector.tensor_copy(cs_sb[:], cs_ps[:, 0:D + 1])
                sv_sb = small.tile([1, D], f32, tag="svsb", bufs=2)
                nc.vector.tensor_copy(sv_sb[:], sv_ps[0:1, 0:D])
                nc.sync.dma_start(cc_in[0:P, :], cs_sb[:, 0:D])
                nc.sync.dma_start(cc_in[P, :], cs_sb[:, D:D + 1])
                nc.sync.dma_start(cc_in[P + 1, :], sv_sb[:])
                nc.gpsimd.collective_compute(
                    "AllReduce",
                    mybir.AluOpType.add,
                    replica_groups=[list(range(NCORES))],
                    ins=[cc_in.opt()],
                    outs=[cc_out.opt()],
                )
                cg = small.tile([P, P], f32, tag="cg", bufs=repeat)
                nc.sync.dma_start(cg[:], cc_out[0:P, :])
                skc = small.tile([P, 1], f32, tag="skc", bufs=repeat)
                nc.sync.dma_start(skc[:], cc_out[P, :])
                svr = small.tile([1, D], f32, tag="svr", bufs=repeat)
                nc.sync.dma_start(svr[:], cc_out[P + 1, :])

                # ============ phase B: ctx / BT / c (all tiny) ============
                # P1_all = (Wk C)^T + s_val bk^T   [j=128, (h e)=128]
                p1_ps = pb_psum.tile([P, P], f32, tag="pb")
                nc.tensor.matmul(p1_ps[:], cg[:], wkt[:], start=True,
                                 stop=False)
                nc.tensor.matmul(p1_ps[:], svr[:], bk_row[:], start=False,
                                 stop=True)
                p1_sb = small.tile([P, P], f32, tag="p1sb", bufs=2)
                nc.vector.tensor_copy(p1_sb[:], p1_ps[:])

                # w = Wk @ s_key + N*bk  (column, all heads)
                w_ps = pb_psum.tile([P, 1], f32, tag="pb")
                nc.tensor.matmul(w_ps[:], wkt[:], skc[:], start=True,
                                 stop=True)
                w_sb = small.tile([P, 1], f32, tag="wsb", bufs=2)
                nc.scalar.add(w_sb[:], w_ps[:], bkn[:])
                wrow_ps = pb_psum.tile([1, P], f32, tag="pb")
                nc.tensor.matmul(wrow_ps[:], w_sb[:], ident[:], start=True,
                                 stop=True)
                wrow = small.tile([1, P], f32, tag="wrow", bufs=2)
                nc.vector.tensor_copy(wrow[:], wrow_ps[:])

                ctx_ps = pb_psum.tile([P, DK], f32, tag="pb")
                for h in range(H):
                    hs = h * DK
                    # ctx_h (T1+T3): P1_h^T @ Wv^T slice -> [e, d] @ part h*32
                    nc.tensor.matmul(ctx_ps[hs:hs + DK, :],
                                     p1_sb[:, hs:hs + DK],
                                     wvt[:, hs:hs + DK], start=True,
                                     stop=False, tile_position=(0, hs))
                    # ctx_h += w_h bv_h^T  (T2+T4)
                    nc.tensor.matmul(ctx_ps[hs:hs + DK, :],
                                     wrow[:, hs:hs + DK],
                                     bv_row[:, hs:hs + DK], start=False,
                                     stop=True, tile_position=(0, hs))

                ctx_sb = small.tile([P, DK], f32, tag="ctxsb", bufs=2)
                nc.scalar.mul(ctx_sb[:], ctx_ps[:], 1.0 / N)
                nc.sync.dma_start(ctx_out[:], ctx_sb[:])

                # BT[i, h*32+d] and bias c[hd]
                bt_ps = pb_psum.tile([P, P], f32, tag="pb")
                c_ps = pb_psum.tile([P, 1], f32, tag="pb")
                for h in range(H):
                    hs = h * DK
                    nc.tensor.matmul(bt_ps[:, hs:hs + DK],
                                     wq_nat[hs:hs + DK, :],
                                     ctx_sb[hs:hs + DK, :], start=True,
                                     stop=True, tile_position=(hs, 0))
                    nc.tensor.matmul(c_ps[hs:hs + DK, :],
                                     ctx_sb[hs:hs + DK, :],
                                     bq_col[hs:hs + DK, :], start=True,
                                     stop=True, tile_position=(hs, hs))
                bt_sb = small.tile([P, P], mmdt, tag="btsb", bufs=2)
                nc.vector.tensor_copy(bt_sb[:], bt_ps[:])
                c_sb = small.tile([P, 1], f32, tag="csb", bufs=2)
                nc.vector.tensor_copy(c_sb[:], c_ps[:])

                # ================= phase C: Y = BT^T q^T + c =================
                for g in range(TILES):
                    qt = q_pool.tile([P, SUB, D], f32, tag="qt")
                    nc.sync.dma_start(
                        qt[:],
                        xq[g * GROUP:(g + 1) * GROUP, :].rearrange(
                            "(s p) i -> p s i", p=P),
                    )
                    qtr_ps = qt_psum.tile([P, GROUP], f32, tag="qtp")
                    for s in range(SUB):
                        nc.tensor.transpose(qtr_ps[:, s * P:(s + 1) * P],
                                            qt[:, s, :], ident[:])
                    qtr = qt_pool.tile([P, GROUP], mmdt, tag="qtr")
                    nc.vector.tensor_copy(qtr[:], qtr_ps[:])
                    yg_ps = y_psum.tile([P, GROUP], f32, tag="yp")
                    nc.tensor.matmul(yg_ps[:], bt_sb[:], qtr[:],
                                     start=True, stop=True)
                    yg = ys_pool.tile([P, GROUP], f32, tag="yg")
                    nc.scalar.add(yg[:], yg_ps[:], c_sb[:])
                    nc.sync.dma_start(
                        y_out[:, g * GROUP:(g + 1) * GROUP], yg[:])

    nc.compile()
    return nc


def _get(repeat=1, use_f32r=True):
    key = (repeat, use_f32r)
    if key not in _cache:
        _cache[key] = _build(repeat=repeat, use_f32r=use_f32r)
    return _cache[key]


def run_spmd(inputs, repeat=1, use_f32r=True, **spmd_kwargs):
    """Shard, run on 8 cores, return BassKernelResults."""
    from concourse.bass_utils import run_bass_kernel_spmd

    nc = _get(repeat=repeat, use_f32r=use_f32r)
    q = np.ascontiguousarray(np.asarray(inputs["query"], dtype=np.float32))
    k = np.ascontiguousarray(np.asarray(inputs["key"], dtype=np.float32))
    v = np.ascontiguousarray(np.asarray(inputs["value"], dtype=np.float32))
    shared = {
        name: np.ascontiguousarray(np.asarray(inputs[name], dtype=np.float32))
        for name in ("Wq", "Wk", "Wv", "bq", "bk", "bv")
    }
    in_maps = []
    for c in range(NCORES):
        sl = slice(c * NS, (c + 1) * NS)
        in_maps.append({
            "xq": q[sl], "xk": k[sl], "xv": v[sl], **shared,
        })
    return run_bass_kernel_spmd(nc, in_maps, list(range(NCORES)),
                                **spmd_kwargs)


def kernel(**inputs):
    res = run_spmd(inputs)
    y_full = np.concatenate(
        [res.results[c]["y"] for c in range(NCORES)], axis=1)
    att_output = y_full.reshape(N, 1, D)
    p_attn = res.results[0]["ctx"].reshape(H, DK, DK)
    return att_output, p_attn


# revision 56
# speedup vs baseline: 3.3857x; 3.3857x over previous
"""Galerkin linear attention (nn_Attention_7172595384411) on 8 TRN2 NeuronCores.

Math (reference):
    q = query @ Wq.T + bq   -> (h, N, dk)   [same for k, v]
    p_attn = einsum("hnd,hne->hde", k, v) / N          (h, dk, dk)
    x      = einsum("hnd,hde->hne", q, p_attn)         (h, N, dk)
    att    = x.transpose(0,2,1).reshape(N, 1, 128)

Key refactor: the projections fold out of the streaming passes.
    p_attn_h = (Wk_h C Wv_h^T + (Wk_h s_key) bv_h^T
                + bk_h (s_val^T Wv_h^T) + N bk_h bv_h^T) / N
with C = key^T value (128x128), s_key = key^T 1, s_val = 1^T value -- all
plain moments of the raw inputs, accumulated per-core over the N-shard and
AllReduced (66KB).  The output pass collapses to ONE fused matmul:
    Y[h*32+d, n] = sum_i BT[i, h*32+d] query[n, i] + c[h*32+d]
where BT[i, h*32+d] = sum_e Wq[h*32+e, i] p_attn_h[e, d] and
c[hd] = bq_h . p_attn_h[:, d].  The row-major flat layout of Y (128, N) is
exactly att_output.

Per core: stream key/value shard (8MB) for moments, AllReduce, tiny on-chip
algebra, stream query shard (4MB) -> Y shard (4MB).  ~16MB DMA/core.
"""

import numpy as np

N = 65536
D = 128
H = 4
DK = 32
NCORES = 8
NS = N // NCORES          # 8192 rows per core
P = 128                   # partitions
SUB = 4                   # 128-row subtiles per DMA tile
GROUP = P * SUB           # 512 rows per tile
TILES = NS // GROUP       # 16 tiles per phase

_cache = {}


def _build(repeat=1, use_f32r=True, use_ar=True, loop_n=1, only="full"):
    import concourse.tile as tile
    from concourse import bacc, mybir
    from concourse.masks import make_identity

    f32 = mybir.dt.float32
    f32r = mybir.dt.float32r
    mmdt = f32r if use_f32r else f32

    def mm_cast(ap):
        # view an f32 DRAM AP as f32r for DMA into an f32r-typed tile
        return ap.bitcast(f32r) if use_f32r else ap

    nc = bacc.Bacc(
        "TRN2",
        target_bir_lowering=False,
        debug=False,
        enable_asserts=True,
        num_devices=NCORES,
    )

    xq = nc.dram_tensor("xq", [NS, D], f32, kind="ExternalInput").ap()
    xk = nc.dram_tensor("xk", [NS, D], f32, kind="ExternalInput").ap()
    xv = nc.dram_tensor("xv", [NS, D], f32, kind="ExternalInput").ap()
    wq = nc.dram_tensor("Wq", [D, D], f32, kind="ExternalInput").ap()
    wk = nc.dram_tensor("Wk", [D, D], f32, kind="ExternalInput").ap()
    wv = nc.dram_tensor("Wv", [D, D], f32, kind="ExternalInput").ap()
    bq = nc.dram_tensor("bq", [D], f32, kind="ExternalInput").ap()
    bk = nc.dram_tensor("bk", [D], f32, kind="ExternalInput").ap()
    bv = nc.dram_tensor("bv", [D], f32, kind="ExternalInput").ap()
    y_out = nc.dram_tensor("y", [D, NS], f32, kind="ExternalOutput").ap()
    ctx_out = nc.dram_tensor("ctx", [D, DK], f32, kind="ExternalOutput").ap()

    from contextlib import ExitStack

    with tile.TileContext(nc) as tc, ExitStack() as es:
        consts = es.enter_context(tc.tile_pool(name="consts", bufs=1))
        kv_pool = es.enter_context(tc.tile_pool(name="kv", bufs=5))
        q_pool = es.enter_context(tc.tile_pool(name="q", bufs=6))
        qt_pool = es.enter_context(tc.tile_pool(name="qt", bufs=3))
        ys_pool = es.enter_context(tc.tile_pool(name="ys", bufs=6))
        small = es.enter_context(tc.tile_pool(name="small", bufs=1))
        acc_psum = es.enter_context(
            tc.tile_pool(name="accp", bufs=1, space="PSUM"))
        pb_psum = es.enter_context(
            tc.tile_pool(name="pbp", bufs=2, space="PSUM"))
        qt_psum = es.enter_context(
            tc.tile_pool(name="qtp", bufs=2, space="PSUM"))
        y_psum = es.enter_context(
            tc.tile_pool(name="yp", bufs=3, space="PSUM"))
        dram = es.enter_context(tc.tile_pool(name="dram", bufs=1,
                                             space="DRAM"))
        if True:
            # ---- constants ----
            ident = consts.tile([P, P], f32)
            make_identity(nc, ident[:])
            ones_f32 = consts.tile([P, SUB, 2], f32)
            nc.gpsimd.memset(ones_f32[:], 1.0)
            ones_row_f32 = consts.tile([1, GROUP], f32)
            nc.gpsimd.memset(ones_row_f32[:], 1.0)
            if use_f32r:
                ones_col = consts.tile([P, 1], mmdt)
                nc.vector.tensor_copy(ones_col[:], ones_f32[:, 0, 0:1])
                ident_r = consts.tile([P, P], mmdt)
                nc.vector.tensor_copy(ident_r[:], ident[:])
                ones_row = consts.tile([1, GROUP], mmdt)
                nc.vector.tensor_copy(ones_row[:], ones_row_f32[:])
            else:
                ones_col = ones_f32[:, 0, 0:1]
                ident_r = ident
                ones_row = ones_row_f32

            wq_nat = consts.tile([P, D], f32)
            nc.sync.dma_start(wq_nat[:], wq[:])
            wk_nat = consts.tile([P, D], f32)
            nc.sync.dma_start(wk_nat[:], wk[:])
            wv_nat = consts.tile([P, D], f32)
            nc.sync.dma_start(wv_nat[:], wv[:])
            bk_row = consts.tile([1, D], f32)
            nc.sync.dma_start(bk_row[:], bk[:])
            bv_row = consts.tile([1, D], f32)
            nc.sync.dma_start(bv_row[:], bv[:])
            bq_col = consts.tile([P, 1], f32)
            nc.sync.dma_start(bq_col[:], bq[:])
            bk_col = consts.tile([P, 1], f32)
            nc.sync.dma_start(bk_col[:], bk[:])

            # transposed weights (one-time)
            wkt_ps = pb_psum.tile([P, P], f32, tag="pb")
            nc.tensor.transpose(wkt_ps[:], wk_nat[:], ident[:])
            wkt = consts.tile([P, P], f32)
            nc.vector.tensor_copy(wkt[:], wkt_ps[:])
            wvt_ps = pb_psum.tile([P, P], f32, tag="pb")
            nc.tensor.transpose(wvt_ps[:], wv_nat[:], ident[:])
            wvt = consts.tile([P, P], f32)
            nc.vector.tensor_copy(wvt[:], wvt_ps[:])

            bkn = consts.tile([P, 1], f32)
            nc.scalar.mul(bkn[:], bk_col[:], float(N))

            import contextlib

            assert loop_n == 1 or (repeat == 1 and not use_ar), (
                "loop_n is a timing-only mode: repeat=1, use_ar=False")
            loop_cm = (tc.For_i(0, loop_n, 1) if loop_n > 1
                       else contextlib.nullcontext())
            es.enter_context(loop_cm)
            for rep in range(repeat):
                last = rep == repeat - 1
                if only == "C":
                    # timing-only: skip phase A/AR/B; fake BT/c from consts
                    bt_sb = wq_nat if not use_f32r else None
                    if use_f32r:
                        bt_sb = consts.tile([P, P], mmdt, name="btfake")
                        nc.vector.tensor_copy(bt_sb[:], wq_nat[:])
                    c_row = consts.tile([1, P], mmdt, name="crfake")
                    nc.vector.tensor_copy(c_row[:], ident[0:1, :])
                    for g in range(TILES):
                        qt = q_pool.tile([P, SUB, D], mmdt, tag="qt")
                        nc.sync.dma_start(
                            qt[:],
                            mm_cast(xq[g * GROUP:(g + 1) * GROUP, :]
                                    .rearrange("(s p) i -> p s i", p=P)))
                        qtr_ps = qt_psum.tile([P, GROUP], mmdt, tag="qtp")
                        for s in range(SUB):
                            nc.tensor.transpose(
                                qtr_ps[:, s * P:(s + 1) * P],
                                qt[:, s, :], ident_r[:])
                        qtr = qt_pool.tile([P, GROUP], mmdt, tag="qtr",
                                           bufs=TILES)
                        nc.vector.tensor_copy(qtr[:], qtr_ps[:])
                        yg_ps = y_psum.tile([P, GROUP], f32, tag="yp")
                        nc.tensor.matmul(yg_ps[:], bt_sb[:], qtr[:],
                                         start=True, stop=False)
                        nc.tensor.matmul(yg_ps[:], c_row[:], ones_row[:],
                                         start=False, stop=True)
                        yg = ys_pool.tile([P, GROUP], f32, tag="yg")
                        nc.vector.tensor_copy(yg[:], yg_ps[:])
                        nc.sync.dma_start(
                            y_out[:, g * GROUP:(g + 1) * GROUP], yg[:])
                    continue

                # ================= phase A: moments =================
                # CS[:, 0:128] = C = key^T value ; CS[:, 128] = s_key
                # SV[0, 0:128] = s_val
                # Phase A in f32r at native 130-col width (even count for the
                # f32r dst rule; 2 cyc/row at mid p-state vs fp32's 4).  The
                # measured HW winner: plain per-subtile m1+m2, no broadcast
                # widening.  C+s_key accumulate in PSUM cols [0:130); s_val
                # in cols [380:510) of the SAME bank (one PSUM bank total).
                vw = D + 2 if use_f32r else D + 1
                acc_ps = acc_psum.tile([P, 512], f32, tag="cs")
                cs_ps = acc_ps[:, 0:vw]
                sv_ps = acc_ps[0:1, 380:380 + vw]

                for t in range(TILES):
                    kt = kv_pool.tile([P, SUB, D], mmdt, tag="kt")
                    nc.sync.dma_start(
                        kt[:],
                        mm_cast(xk[t * GROUP:(t + 1) * GROUP, :].rearrange(
                            "(s p) i -> p s i", p=P)),
                    )
                    vt = kv_pool.tile([P, SUB, vw], mmdt, tag="vt")
                    nc.sync.dma_start(
                        vt[:, :, 0:D],
                        mm_cast(xv[t * GROUP:(t + 1) * GROUP, :].rearrange(
                            "(s p) i -> p s i", p=P)),
                    )
                    if use_f32r:
                        nc.vector.tensor_copy(vt[:, :, D:vw], ones_f32[:])
                    else:
                        nc.gpsimd.memset(vt[:, :, D:vw], 1.0)

                    for s in range(SUB if only != "Adma" else 0):
                        first = t == 0 and s == 0
                        fin = t == TILES - 1 and s == SUB - 1
                        nc.tensor.matmul(
                            cs_ps[:],
                            kt[:, s, :],
                            vt[:, s, :],
                            start=(True if only == "Anoacc" else first),
                            stop=(True if only == "Anoacc" else fin),
                            skip_group_check=True,
                        )
                        # start=False always: the first m1's start=True
                        # zeroes the whole 2KB PSUM row, incl. this region.
                        if only != "Anom2":
                            nc.tensor.matmul(
                                sv_ps[:],
                                ones_col[:],
                                vt[:, s, :],
                                start=False,
                                stop=fin,
                                skip_group_check=True,
                            )

                if only == "Adma":
                    continue

                # ================= AllReduce (starts ASAP) =================
                cc_in = dram.tile([130, D], f32, tag="ccin")
                cc_out = dram.tile([130, D], f32, addr_space="Shared",
                                   tag="ccout")
                cs_sb = small.tile([P, D + 1], f32, tag="cssb", bufs=2)
                nc.vector.tensor_copy(cs_sb[:], acc_ps[:, 0:D + 1])
                sv_sb = small.tile([1, D], f32, tag="svsb", bufs=2)
                nc.vector.tensor_copy(sv_sb[:], acc_ps[0:1, 380:380 + D])
                nc.sync.dma_start(cc_in[0:P, :], cs_sb[:, 0:D])
                nc.sync.dma_start(cc_in[P, :], cs_sb[:, D:D + 1])
                nc.sync.dma_start(cc_in[P + 1, :], sv_sb[:])
                if use_ar:
                    nc.gpsimd.collective_compute(
                        "AllReduce",
                        mybir.AluOpType.add,
                        replica_groups=[list(range(NCORES))],
                        ins=[cc_in.opt()],
                        outs=[cc_out.opt()],
                    )
                else:  # timing diagnostics only (wrong numerics)
                    nc.sync.dma_start(cc_out[:], cc_in[:])

                # ===== phase C1: load+transpose ALL of q during the AR =====
                # whole transposed q shard stays in SBUF (16 x 2KB/partition)
                qtrs = []
                for g in range(0 if only.startswith("A") else TILES):
                    qt = q_pool.tile([P, SUB, D], mmdt, tag="qt")
                    nc.sync.dma_start(
                        qt[:],
                        mm_cast(xq[g * GROUP:(g + 1) * GROUP, :].rearrange(
                            "(s p) i -> p s i", p=P)),
                    )
                    qtr_ps = qt_psum.tile([P, GROUP], mmdt, tag="qtp")
                    for s in range(SUB):
                        nc.tensor.transpose(qtr_ps[:, s * P:(s + 1) * P],
                                            qt[:, s, :], ident_r[:])
                    qtr = qt_pool.tile([P, GROUP], mmdt, tag="qtr",
                                       bufs=TILES)
                    nc.vector.tensor_copy(qtr[:], qtr_ps[:])
                    qtrs.append(qtr)

                cg = small.tile([P, P], f32, tag="cg", bufs=2)
                nc.sync.dma_start(cg[:], cc_out[0:P, :])
                skc = small.tile([P, 1], f32, tag="skc", bufs=2)
                nc.sync.dma_start(skc[:], cc_out[P, :])
                svr = small.tile([1, D], f32, tag="svr", bufs=2)
                nc.sync.dma_start(svr[:], cc_out[P + 1, :])

                # ============ phase B: ctx / BT / c (all tiny) ============
                # P1_all = (Wk C)^T + s_val bk^T   [j=128, (h e)=128]
                p1_ps = pb_psum.tile([P, P], f32, tag="pb")
                nc.tensor.matmul(p1_ps[:], cg[:], wkt[:], start=True,
                                 stop=False)
                nc.tensor.matmul(p1_ps[:], svr[:], bk_row[:], start=False,
                                 stop=True)
                p1_sb = small.tile([P, P], f32, tag="p1sb", bufs=2)
                nc.vector.tensor_copy(p1_sb[:], p1_ps[:])

                # w = Wk @ s_key + N*bk  (column, all heads)
                w_ps = pb_psum.tile([P, 1], f32, tag="pb")
                nc.tensor.matmul(w_ps[:], wkt[:], skc[:], start=True,
                                 stop=True)
                w_sb = small.tile([P, 1], f32, tag="wsb", bufs=2)
                nc.vector.tensor_add(w_sb[:], w_ps[:], bkn[:])
                wrow_ps = pb_psum.tile([1, P], f32, tag="pb")
                nc.tensor.matmul(wrow_ps[:], w_sb[:], ident[:], start=True,
                                 stop=True)
                wrow = small.tile([1, P], f32, tag="wrow", bufs=2)
                nc.vector.tensor_copy(wrow[:], wrow_ps[:])

                ctx_ps = pb_psum.tile([P, DK], f32, tag="pb")
                for h in range(H):
                    hs = h * DK
                    # ctx_h (T1+T3): P1_h^T @ Wv^T slice -> [e, d] @ part h*32
                    nc.tensor.matmul(ctx_ps[hs:hs + DK, :],
                                     p1_sb[:, hs:hs + DK],
                                     wvt[:, hs:hs + DK], start=True,
                                     stop=False, tile_position=(0, hs))
                    # ctx_h += w_h bv_h^T  (T2+T4)
                    nc.tensor.matmul(ctx_ps[hs:hs + DK, :],
                                     wrow[:, hs:hs + DK],
                                     bv_row[:, hs:hs + DK], start=False,
                                     stop=True, tile_position=(0, hs))

                ctx_sb = small.tile([P, DK], f32, tag="ctxsb", bufs=2)
                nc.vector.tensor_scalar_mul(ctx_sb[:], ctx_ps[:], 1.0 / N)
                nc.sync.dma_start(ctx_out[:], ctx_sb[:])

                # BT[i, h*32+d] and bias c[hd]
                bt_ps = pb_psum.tile([P, P], f32, tag="pb")
                c_ps = pb_psum.tile([P, 1], f32, tag="pb")
                for h in range(H):
                    hs = h * DK
                    nc.tensor.matmul(bt_ps[:, hs:hs + DK],
                                     wq_nat[hs:hs + DK, :],
                                     ctx_sb[hs:hs + DK, :], start=True,
                                     stop=True, tile_position=(hs, 0))
                    nc.tensor.matmul(c_ps[hs:hs + DK, :],
                                     ctx_sb[hs:hs + DK, :],
                                     bq_col[hs:hs + DK, :], start=True,
                                     stop=True, tile_position=(hs, hs))
                bt_sb = small.tile([P, P], mmdt, tag="btsb", bufs=2)
                nc.vector.tensor_copy(bt_sb[:], bt_ps[:])
                c_sb = small.tile([P, 1], f32, tag="csb", bufs=2)
                nc.vector.tensor_copy(c_sb[:], c_ps[:])
                crow_ps = pb_psum.tile([1, P], f32, tag="pb")
                nc.tensor.matmul(crow_ps[:], c_sb[:], ident[:], start=True,
                                 stop=True)
                c_row = small.tile([1, P], mmdt, tag="crow", bufs=2)
                nc.vector.tensor_copy(c_row[:], crow_ps[:])

                # ============ phase C2: Y = BT^T q^T + c 1^T, store ============
                for g in range(0 if only.startswith("A") else TILES):
                    yg_ps = y_psum.tile([P, GROUP], f32, tag="yp")
                    nc.tensor.matmul(yg_ps[:], bt_sb[:], qtrs[g][:],
                                     start=True, stop=False)
                    # bias via K=1 outer product accumulated on PE
                    nc.tensor.matmul(yg_ps[:], c_row[:], ones_row[:],
                                     start=False, stop=True)
                    yg = ys_pool.tile([P, GROUP], f32, tag="yg")
                    nc.vector.tensor_copy(yg[:], yg_ps[:])
                    nc.sync.dma_start(
                        y_out[:, g * GROUP:(g + 1) * GROUP], yg[:])

    nc.compile()
    return nc


def _get(repeat=1, use_f32r=True, use_ar=True, loop_n=1, only="full"):
    key = (repeat, use_f32r, use_ar, loop_n, only)
    if key not in _cache:
        _cache[key] = _build(repeat=repeat, use_f32r=use_f32r, use_ar=use_ar,
                             loop_n=loop_n, only=only)
    return _cache[key]


def run_spmd(inputs, repeat=1, use_f32r=True, **spmd_kwargs):
    """Shard, run on 8 cores, return BassKernelResults."""
    from concourse.bass_utils import run_bass_kernel_spmd

    nc = _get(repeat=repeat, use_f32r=use_f32r)
    q = np.ascontiguousarray(np.asarray(inputs["query"], dtype=np.float32))
    k = np.ascontiguousarray(np.asarray(inputs["key"], dtype=np.float32))
    v = np.ascontiguousarray(np.asarray(inputs["value"], dtype=np.float32))
    shared = {
        name: np.ascontiguousarray(np.asarray(inputs[name], dtype=np.float32))
        for name in ("Wq", "Wk", "Wv", "bq", "bk", "bv")
    }
    in_maps = []
    for c in range(NCORES):
        sl = slice(c * NS, (c + 1) * NS)
        in_maps.append({
            "xq": q[sl], "xk": k[sl], "xv": v[sl], **shared,
        })
    return run_bass_kernel_spmd(nc, in_maps, list(range(NCORES)),
                                **spmd_kwargs)


def kernel(**inputs):
    # Occasional transient device faults can yield a garbage first run;
    # every core holds the same AllReduced ctx, so cross-core agreement
    # (plus finiteness) is a cheap integrity check worth one retry.
    res = None
    for _ in range(3):
        res = run_spmd(inputs)
        ctx0 = res.results[0]["ctx"]
        ok = bool(np.isfinite(ctx0).all()) and float(
            np.abs(ctx0).max()) > 0.0
        if ok:
            scale = max(1.0, float(np.abs(ctx0).max()))
            dev = max(
                float(np.abs(res.results[c]["ctx"] - ctx0).max())
                for c in range(1, NCORES))
            ok = dev <= 1e-4 * scale
        if ok and all(
                np.isfinite(res.results[c]["y"]).all()
                for c in range(NCORES)):
            break
    y_full = np.concatenate(
        [res.results[c]["y"] for c in range(NCORES)], axis=1)
    att_output = y_full.reshape(N, 1, D)
    p_attn = res.results[0]["ctx"].reshape(H, DK, DK)
    return att_output, p_attn
